# revision 25
# baseline (speedup 1.0000x reference)
"""ExternalAttention Trainium2 Bass kernel (bf16 I/O, transposed layout,
row-packed softmax).

Math (per batch b, with N = H*W = 4096 tokens, C = 512, K = 64):
    x      = inputs @ w1 + b1          [N, C]
    logits = x @ m0                    [N, K]
    attn   = softmax(logits, axis=N)   (the L1-normalize over N afterwards is a
                                        divide by 1 + 1e-9 -> skipped; the max
                                        subtraction is shift-invariant and
                                        logits are O(1) -> skipped)
    y      = attn @ m1 @ w2            [N, C]
    out    = relu(BN_affine(y) + inputs)

Host-side folds (all tiny C x C / C x K matrices):
    wm    = w1 @ m0                                 [C, K]  (b1 @ m0 shifts each
            softmax column by a constant -> softmax-invariant, dropped)
    scale = gamma / sqrt(bn_var + eps); shift = beta - bn_mean * scale
    w2m   = m1 @ (w2 * scale)                       [K, C]
    => out = relu(colsoftmax(inputs @ wm) @ w2m + shift + inputs)

The kernel is PE/HBM-balanced, so everything is stored bf16 (rel-err budget
2e-2, bf16 contributes ~4e-3) and the host pre-transposes inputs to x^T
[C, N] per batch so no PE transposes / psum copies are needed on device.

Device kernel (per core, 2 batches, data-parallel over B=16 on 8 cores):
    - loads: c4-major [128, 2048] bf16 half-tiles on the sync HWDGE ring
    - mm1: chunk PAIRS share one [128, 512] psum tile - chunk 2p in psum
      rows 0:64, chunk 2p+1 in rows 64:128 (PE col-group positioning), so
      each ACT exp covers two chunks at full 128-lane width with one
      accumulated row-sum column; K=64 exactly fills both halves
    - softmax totals: per-half sums are added across the partition halves
      with one tiny P2 = [[I,I],[I,I]]-weighted matmul, then reciprocal;
      the normalization is folded into a duplicated-row w2m copy (one
      [128, 512] DVE scale)
    - mm2 (per half, per c4): 4 residual start-matmuls I^T @ xT (no chain
      dependency), then 4 w2m_s^T @ attn stop-matmuls using 64-row
      contraction on the matching array half; relu+BN-shift drains psum ->
      bf16 in-place into the xT tiles (ACT relu with per-partition bias /
      DVE add+max two-op), one [128, 2048] store per group on the sync ring
      (FIFO behind the loads, so loads stream un-contended)
    - phase order b0mm1, b0mm2(h0), b1mm1, b0mm2(h1), b1mm2 keeps the PE
      busy across both softmax dependency chains
    - 12 zero-weight warmup matmuls at t0 (no DMA dependency) lift the PE
      HAM clock gate 1.2 -> 2.4 GHz before the first data lands
"""

import os
import sys
from contextlib import ExitStack

import numpy as np
import ml_dtypes

for _p in ("/opt/trn_rl_repo", os.path.expanduser("~/.axon_site/_ro/trn_rl_repo")):
    if os.path.isdir(_p) and _p not in sys.path:
        sys.path.insert(0, _p)

import concourse.bass as bass
import concourse.mybir as mybir
import concourse.tile as tile
from concourse import bacc
from concourse.bass import ts
from concourse.bass_utils import run_bass_kernel_spmd

B, H, W, C, K = 16, 64, 64, 512, 64
N = H * W  # 4096 tokens
BN_EPS = 1e-3
NCORES = 8
BPC = B // NCORES  # batches per core = 2
NCHUNK = 8  # 512-token chunks per batch
NPAIR = 4   # chunk pairs per batch

F32 = mybir.dt.float32
BF16 = mybir.dt.bfloat16
NPBF16 = ml_dtypes.bfloat16

_cached_nc = None


def _build_nc() -> bass.Bass:
    nc = bacc.Bacc(None, target_bir_lowering=False, debug=False)
    x = nc.dram_tensor("x", [BPC, C, N], BF16, kind="ExternalInput")
    wm = nc.dram_tensor("wm", [128, 4 * K], BF16, kind="ExternalInput")
    cbf = nc.dram_tensor("cbf", [128, 256 + C], BF16, kind="ExternalInput")
    cf32 = nc.dram_tensor("cf32", [128, 132], F32, kind="ExternalInput")
    y = nc.dram_tensor("y", [BPC, C, N], BF16, kind="ExternalOutput")

    with tile.TileContext(nc) as tc, ExitStack() as ctx:
        const = ctx.enter_context(tc.tile_pool(name="const", bufs=1))
        xt_pool = ctx.enter_context(tc.tile_pool(name="xt", bufs=2 * 4))
        attn_pool = ctx.enter_context(tc.tile_pool(name="attn", bufs=2))
        small = ctx.enter_context(tc.tile_pool(name="small", bufs=2))

        cbf_sb = const.tile([128, 256 + C], BF16)  # [I | P2 | w2m2]
        wm_sb = const.tile([128, 4, K], BF16)   # [p, c4, k] = wm[c4*128+p, k]
        cf32_sb = const.tile([128, 132], F32)   # [shift c4-major | P2 f32]
        zw_sb = const.tile([128, 128], BF16)    # zero warmup weights
        warm_sb = const.tile([128, 512], BF16)  # zero warmup rhs

        xv = [x[b].rearrange("(c4 p) n -> c4 p n", p=128) for b in range(BPC)]
        yv = [y[b].rearrange("(c4 p) n -> c4 p n", p=128) for b in range(BPC)]

        xts, attns, sums_t, sums_bf, w2ms = [], [], [], [], []
        for b in range(BPC):
            xts.append([
                xt_pool.tile([128, N], BF16, tag="xt", name=f"xt{b}_{c4}")
                for c4 in range(4)
            ])
            # packed: chunk 2p in rows 0:64, chunk 2p+1 in rows 64:128
            attns.append(attn_pool.tile([128, N // 2], BF16, tag="attn",
                                        name=f"attn{b}"))
            sums_t.append(small.tile([128, NPAIR], F32, tag="sums",
                                     name=f"sums{b}"))
            sums_bf.append(small.tile([128, NPAIR], BF16, tag="sumsb",
                                      name=f"sumsb{b}"))
            w2ms.append(small.tile([128, C], BF16, tag="w2ms",
                                   name=f"w2ms{b}"))

        # warmup weights come from memsets so the PE can start with zero
        # DMA dependency; the first critical x half-tile leads the ring and
        # the tiny wm rides just behind it
        nc.vector.memset(zw_sb, 0.0)
        nc.vector.memset(warm_sb, 0.0)

        def load_half(b, h, c4s=range(4)):
            hs = ts(h, N // 2)
            for c4 in c4s:
                nc.sync.dma_start(out=xts[b][c4][:, hs], in_=xv[b][c4][:, hs])

        load_half(0, 0, c4s=[0, 1])
        # wm rides here: lands before the first mm1 matmul without delaying
        # the exp-gating c4-3 half-tile
        nc.sync.dma_start(
            out=wm_sb, in_=wm.rearrange("p (c4 k) -> p c4 k", c4=4))
        load_half(0, 0, c4s=[2, 3])
        load_half(0, 1)
        # merged consts are first needed ~25us in (first mm2 / chain);
        # riding here keeps them off the critical b0/b1 load paths
        nc.sync.dma_start(out=cbf_sb, in_=cbf[:, :])
        nc.sync.dma_start(out=cf32_sb, in_=cf32[:, :])
        load_half(1, 0)
        load_half(1, 1)

        ident_sb = cbf_sb[:, 0:128]
        p2_sb = cbf_sb[:, 128:256]
        w2m2_sb = cbf_sb[:, 256:256 + C]
        shift_sb = cf32_sb[:, 0:4]
        p2f_sb = cf32_sb[:, 4:132]

        def mm1_pair(ps, b, p, fill=False):
            """One packed chunk-pair (rows 0:64 / 64:128 of one psum tile
            via PE column groups) + a full-width exp."""
            t = ps.tile([128, 512], F32, tag="ps", name=f"l{b}_{p}")
            for c4 in range(4):
                nc.tensor.matmul(
                    t[0:K], lhsT=wm_sb[:, c4],
                    rhs=xts[b][c4][:, ts(2 * p, 512)],
                    start=(c4 == 0), stop=(c4 == 3),
                    skip_group_check=True)
                nc.tensor.matmul(
                    t[K:128], lhsT=wm_sb[:, c4],
                    rhs=xts[b][c4][:, ts(2 * p + 1, 512)],
                    start=(c4 == 0), stop=(c4 == 3),
                    skip_group_check=True)
                if fill:
                    # keep the HAM clock hot through the load-gated
                    # stretch with zero-weight filler matmuls
                    for _ in range(4):
                        nc.tensor.matmul(wp, lhsT=zw_sb, rhs=warm_sb,
                                         start=True, stop=True,
                                         skip_group_check=True)
            # one exp covers both chunks; row sums -> sums[:, p]
            nc.scalar.activation(
                out=attns[b][:, ts(p, 512)], in_=t,
                func=mybir.ActivationFunctionType.Exp,
                accum_out=sums_t[b][:, p:p + 1],
            )

        def chain(ps, b):
            """Fold column-softmax normalization into the w2m rows."""
            # per-k totals need rows k and k+64 added: one P2=[[I,I],[I,I]]
            # matmul does the cross-half add, duplicated into both halves.
            # The bf16 staging copy runs on ACT (same engine as the accum
            # writes -> FIFO-ordered, race-free) and keeps the P2 weight
            # load bf16-fast instead of the ~1us fp32 LOW_HIGH stall.
            nc.scalar.copy(sums_bf[b], sums_t[b])
            pt = ps.tile([128, 512], F32, tag="ps", name=f"tot{b}")
            nc.tensor.matmul(pt[:, 0:NPAIR], lhsT=p2_sb, rhs=sums_bf[b],
                             start=True, stop=True, skip_group_check=True)
            total = small.tile([128, 1], F32, tag="tot")
            nc.vector.reduce_sum(out=total, in_=pt[:, 0:NPAIR],
                                 axis=mybir.AxisListType.X)
            rsum = small.tile([128, 1], F32, tag="rs")
            nc.vector.reciprocal(out=rsum, in_=total)
            nc.vector.tensor_scalar_mul(w2ms[b], w2m2_sb, rsum)

        def mm2_group(ps, b, h, c4, engines, fine_stores=False, qs=None):
            """One (half, c4) output group: 4 residual matmuls + 4 attn
            matmuls into psum, relu+shift drains, one [128, 2048] store."""
            if qs is None:
                qs = [4 * h + i for i in range(4)]
            tiles = [ps.tile([128, 512], F32, tag="ps", name=f"y{b}_{c4}_{q}")
                     for q in qs]
            sh = shift_sb[:, c4:c4 + 1]
            for t, q in zip(tiles, qs):
                # residual first: psum = I^T @ xT chunk (no chain dependency);
                # the two 64-col halves run concurrently on disjoint PE
                # column groups
                for ch in range(2):
                    nc.tensor.matmul(t[64 * ch:64 * ch + 64],
                                     lhsT=ident_sb[:, 64 * ch:64 * ch + 64],
                                     rhs=xts[b][c4][:, ts(q, 512)],
                                     start=True, stop=False,
                                     skip_group_check=True)
            for t, q in zip(tiles, qs):
                rh = K * (q % 2)
                for ch in range(2):
                    cs = 128 * c4 + 64 * ch
                    nc.tensor.matmul(t[64 * ch:64 * ch + 64],
                                     lhsT=w2ms[b][rh:rh + K, cs:cs + 64],
                                     rhs=attns[b][rh:rh + K, ts(q // 2, 512)],
                                     start=False, stop=True,
                                     skip_group_check=True)
            for t, q, eng in zip(tiles, qs, engines):
                if eng == "a":
                    nc.scalar.activation(
                        out=xts[b][c4][:, ts(q, 512)], in_=t,
                        func=mybir.ActivationFunctionType.Relu, bias=sh)
                else:
                    nc.vector.tensor_scalar(
                        out=xts[b][c4][:, ts(q, 512)], in0=t,
                        scalar1=sh, scalar2=0.0,
                        op0=mybir.AluOpType.add, op1=mybir.AluOpType.max)
            if fine_stores:
                rings = {4: nc.sync, 5: nc.gpsimd, 6: nc.gpsimd,
                         7: nc.scalar}
                for q in qs:
                    rings.get(q, nc.sync).dma_start(
                        out=yv[b][c4][:, ts(q, 512)],
                        in_=xts[b][c4][:, ts(q, 512)])
            else:
                nc.sync.dma_start(out=yv[b][c4][:, ts(h, N // 2)],
                                  in_=xts[b][c4][:, ts(h, N // 2)])

        with tc.tile_pool(name="ps", bufs=7, space="PSUM") as ps, \
             tc.tile_pool(name="wps", bufs=1, space="PSUM") as wps:
            # PE warmup on zero weights: lift the HAM clock gate while the
            # first loads land
            wp = wps.tile([128, 512], F32, tag="w", name="warm")
            for i in range(12):
                nc.tensor.matmul(wp, lhsT=zw_sb, rhs=warm_sb,
                                 start=(i == 0), stop=(i == 11),
                                 skip_group_check=True)

            def g(b, h, c4, **kw):
                mm2_group(ps, b, h, c4, "avav" if c4 % 2 == 0 else "vava",
                          **kw)

            for p in range(NPAIR):
                # fill only the load-gated first pair of each half
                mm1_pair(ps, 0, p, fill=(p % 2 == 0))
            chain(ps, 0)
            for c4 in range(4):
                g(0, 0, c4)
            for p in range(NPAIR):
                mm1_pair(ps, 1, p)
            for c4 in range(4):
                g(0, 1, c4)
            chain(ps, 1)
            for c4 in range(4):
                g(1, 0, c4)
            for c4 in range(3):
                g(1, 1, c4)
            mm2_group(ps, 1, 1, 3, "va", fine_stores=True, qs=[4, 5])
            mm2_group(ps, 1, 1, 3, "av", fine_stores=True, qs=[6, 7])

    nc.finalize()
    return nc


def _get_nc() -> bass.Bass:
    global _cached_nc
    if _cached_nc is None:
        _cached_nc = _build_nc()
    return _cached_nc


def _fold_weights(w1, m0, m1, w2, gamma, beta, bn_mean, bn_var):
    w1 = np.asarray(w1, np.float64)
    m0 = np.asarray(m0, np.float64)
    m1 = np.asarray(m1, np.float64)
    w2 = np.asarray(w2, np.float64)
    gamma = np.asarray(gamma, np.float64)
    beta = np.asarray(beta, np.float64)
    bn_mean = np.asarray(bn_mean, np.float64)
    bn_var = np.asarray(bn_var, np.float64)

    wm = (w1 @ m0).astype(np.float32)  # [C, K]
    scale = gamma / np.sqrt(bn_var + BN_EPS)
    w2m = (m1 @ (w2 * scale[None, :])).astype(np.float32)  # [K, C]
    shift = (beta - bn_mean * scale).astype(np.float32)  # [C]
    return wm, w2m, shift


def _run(inputs_np: dict, trace: bool = False):
    nc = _get_nc()
    inp = np.asarray(inputs_np["inputs"], np.float32).reshape(B, N, C)
    # transposed bf16 layout [B, C, N] so device DMAs are contiguous
    xt = inp.transpose(0, 2, 1).astype(NPBF16)
    wm, w2m, shift = _fold_weights(
        inputs_np["w1"], inputs_np["m0"], inputs_np["m1"], inputs_np["w2"],
        inputs_np["gamma"], inputs_np["beta"],
        inputs_np["bn_mean"], inputs_np["bn_var"],
    )
    # pre-swizzle wm rows to [p, c4*k] so the const DMA is contiguous
    wm_sw = np.ascontiguousarray(
        wm.reshape(4, 128, K).transpose(1, 0, 2)
    ).reshape(128, 4 * K).astype(NPBF16)
    w2m2 = np.concatenate([w2m, w2m], axis=0)  # [128, C]
    eye = np.eye(128, dtype=np.float32)
    p2 = np.tile(np.eye(64, dtype=np.float32), (2, 2))  # [[I,I],[I,I]]
    cbf = np.concatenate([eye, p2, w2m2], axis=1).astype(NPBF16)
    shift_sw = shift.reshape(4, 128).T  # [128, 4] f32
    cf32 = np.ascontiguousarray(
        np.concatenate([shift_sw, p2], axis=1).astype(np.float32))
    in_maps = [
        {
            "x": np.ascontiguousarray(xt[i * BPC:(i + 1) * BPC]),
            "wm": wm_sw,
            "cbf": cbf,
            "cf32": cf32,
        }
        for i in range(NCORES)
    ]
    res = run_bass_kernel_spmd(nc, in_maps, core_ids=list(range(NCORES)),
                               trace=trace)
    out = np.concatenate([r["y"] for r in res.results], axis=0)  # [B, C, N]
    out = out.astype(np.float32).transpose(0, 2, 1).reshape(B, H, W, C)
    return np.ascontiguousarray(out), res


def kernel(**inputs) -> np.ndarray:
    out, _ = _run(inputs, trace=False)
    return out
